# revision 7
# baseline (speedup 1.0000x reference)
"""Trainium2 Bass kernel for nn_Decoder_27848567948011.

Sequential LSTM-style decoder, 40 steps, batch 8192, data-parallel over 8 cores.
Feature-major on-chip layout: activations stored as [feature, batch_tile].

Scale folding (avoids explicit sigmoid; single ACT table set `exp_and_others`):
  sigmoid(x) = 0.5*(tanh(x/2)+1)
  stored TF = tanh(f/2) etc.;  C~ = 2*c_new;  H~ = 2*h_new;  D~ = lam~*C~
  U = (TF+1)*D~_prev ; V = (TI+1)*G ; C~ = U+V ; H~ = (TO+1)*tanh(C~/2)
  lam~ = intensity/2; weights consuming h_new or c_new are pre-halved on host.

256-wide tensors (gates chunks, C~, H~, D~) are stored as [128, 2*NB] tiles
(chunk0 | chunk1 along free dim) so ACT/DVE ops run at FD=1024.

Self-contained: hardcodes shapes; reads nothing from the problem directory.
"""
import numpy as np

B, H, F, Z, T = 8192, 256, 128, 64, 43
STEPS = T - 3
N_CORES = 8
NBC = B // N_CORES          # batch per core (1024)
NT = 2                      # batch tiles per core
NB = NBC // NT              # batch tile size (512)
HC = H // 128               # partition chunks for 256-dim (2)

DT_MM = "bfloat16"          # matmul dtype
CELL_DT = "bfloat16"        # elementwise cell dtype
REPEAT = 1                  # >1: run the whole scan REPEAT times (timing builds)
_cache = {}


def _build_kernel(dt_mm_name: str, cell_dt_name: str, repeat: int):
    import concourse.bass as bass
    import concourse.tile as tile
    from concourse import bacc, mybir

    f32 = mybir.dt.float32
    dt_mm = getattr(mybir.dt, dt_mm_name)
    dt_c = getattr(mybir.dt, cell_dt_name)
    AF = mybir.ActivationFunctionType
    OP = mybir.AluOpType

    nc = bacc.Bacc("TRN2", target_bir_lowering=False, debug=False, num_devices=N_CORES)

    din = {}
    def inp(name, shape, dt):
        din[name] = nc.dram_tensor(name, shape, dt, kind="ExternalInput")
        return din[name]

    epsT = inp("epsT", (STEPS, 128, NBC), dt_c)     # [eps_inf; eps_prior] fmajor
    lamT = inp("lamT", (STEPS, NBC), dt_c)          # lam~ = intensity/2
    Wxz = inp("Wxz", (Z, 4 * H), dt_mm)
    Wxy = inp("Wxy", (F, 4 * H), dt_mm)
    Whh = inp("Whh", (H, 4 * H), dt_mm)             # Wh / 2
    w1h = inp("w1h", (H, F), dt_mm)                 # w1 / 2
    w2d = inp("w2", (F, F), dt_mm)
    w3d = inp("w3", (F, F), dt_mm)
    W47c = inp("W47c", (H, 128), dt_mm)             # [w4_c | w7_c] / 2
    W4y = inp("W4y", (F, Z), dt_mm)
    W47yp = inp("W47yp", (F, 128), dt_mm)
    W58 = inp("W58", (128, 128), dt_mm)
    W69 = inp("W69", (128, 128), dt_mm)
    ident = inp("ident", (128, 128), dt_mm)
    baseg = inp("baseg", (4 * H, NBC), dt_mm)       # (h_i@Wx[64:320] + b_lstm).T
    base47 = inp("base47", (128, NBC), dt_mm)
    bvec = inp("bvec", (128, 6), f32)   # b1|b2|b3|[b5;b8]|0.5[b6;b9]|[b6;b9]

    dys = nc.dram_tensor("ys", (STEPS, F, NBC), f32, kind="ExternalOutput")
    dmeans = nc.dram_tensor("means", (STEPS, Z, NBC), f32, kind="ExternalOutput")
    dlvs = nc.dram_tensor("lvs", (STEPS, Z, NBC), f32, kind="ExternalOutput")
    dzs = nc.dram_tensor("zs", (STEPS, Z, NBC), f32, kind="ExternalOutput")
    dzps = nc.dram_tensor("zps", (STEPS, Z, NBC), f32, kind="ExternalOutput")

    # gate pair -> (m chunks, ACT input scale)
    PAIRS = [("i", (0, 1), 0.5), ("f", (2, 3), 0.5), ("g", (4, 5), 1.0),
             ("o", (6, 7), 0.5)]

    with tile.TileContext(nc) as tc:
        with (
            tc.tile_pool(name="wpool", bufs=1) as wpool,
            tc.tile_pool(name="gates", bufs=1) as gpool,
            tc.tile_pool(name="cellp", bufs=1) as cpool,
            tc.tile_pool(name="carry", bufs=2) as carry,
            tc.tile_pool(name="iop", bufs=3) as iop,
            tc.tile_pool(name="psum", bufs=1, space="PSUM") as psum,
        ):
            def load(name):
                d = din[name]
                t = wpool.tile(list(d.shape), d.dtype, name=f"sb_{name}")
                nc.sync.dma_start(t[:], d[:])
                return t

            def load_rows(name, nchunk):
                d = din[name]
                ts = []
                for k in range(nchunk):
                    t = wpool.tile([128, d.shape[1]], d.dtype, name=f"sb_{name}{k}")
                    nc.sync.dma_start(t[:], d[k * 128:(k + 1) * 128, :])
                    ts.append(t)
                return ts

            s_Wxz = load("Wxz"); s_Wxy = load("Wxy")
            s_Wh = load_rows("Whh", HC)
            s_w1 = load_rows("w1h", HC)
            s_w2 = load("w2"); s_w3 = load("w3")
            s_W47c = load_rows("W47c", HC)
            s_W4y = load("W4y"); s_W47yp = load("W47yp")
            s_W58 = load("W58"); s_W69 = load("W69")
            s_I = load("ident")
            s_base = load_rows("baseg", 8)
            s_base47 = load("base47")
            s_bv = load("bvec")

            z_prev = [None] * NT   # [128,NB] tile, rows 0:Z hold z (dtype dt_mm-ready)
            y_prev = [None] * NT   # [128,NB] dt_mm
            h_prev = [None] * NT   # [128,2NB] dt_c/dt_mm (chunk0|chunk1)
            d_prev = [None] * NT   # [128,2NB]

            for rep in range(repeat):
              for j in range(STEPS):
                first = (j == 0)
                for nt in range(NT):
                    ns = slice(nt * NB, (nt + 1) * NB)
                    eps = iop.tile([128, NB], dt_c, name="eps", tag=f"eps{nt}")
                    nc.sync.dma_start(eps[:], epsT[j, :, ns])
                    lam = iop.tile([128, NB], dt_c, name="lam", tag=f"lam{nt}")
                    lrow = lamT[j, ns]
                    lam_src = bass.AP(tensor=lrow.tensor, offset=lrow.offset,
                                      ap=[[0, 128]] + list(lrow.ap))
                    nc.sync.dma_start(lam[:], lam_src)

                    # ---- gates: 4 pairs, each a [128, 2NB] psum (2 banks) ----
                    gt = {}
                    for pname, (m0, m1), scale in PAIRS:
                        g = gpool.tile([128, 2 * NB], dt_c, name="g",
                                       tag=f"g{pname}_{nt}")
                        if not first:
                            ps = psum.tile([128, 2 * NB], f32, name="gps",
                                           tag="gps", bufs=3)
                            for ci, m in enumerate((m0, m1)):
                                out = ps[:, ci * NB:(ci + 1) * NB]
                                mm = slice(m * 128, (m + 1) * 128)
                                # z contribution last: z_prev is the longest
                                # dependency chain from the previous step.
                                nc.tensor.matmul(out, s_I[:], s_base[m][:, ns],
                                                 start=True, stop=False)
                                nc.tensor.matmul(out, s_Wxy[:, mm], y_prev[nt][:],
                                                 start=False, stop=False)
                                for k in range(HC):
                                    nc.tensor.matmul(
                                        out, s_Wh[k][:, mm],
                                        h_prev[nt][:, k * NB:(k + 1) * NB],
                                        start=False, stop=False)
                                nc.tensor.matmul(out, s_Wxz[:, mm],
                                                 z_prev[nt][0:Z, :],
                                                 start=False, stop=True)
                            nc.scalar.activation(g[:], ps[:], AF.Tanh, scale=scale)
                        else:
                            for ci, m in enumerate((m0, m1)):
                                nc.scalar.activation(g[:, ci * NB:(ci + 1) * NB],
                                                     s_base[m][:, ns], AF.Tanh,
                                                     scale=scale)
                        gt[pname] = g
                    TI, TF, TG, TO = gt["i"], gt["f"], gt["g"], gt["o"]

                    # ---- cell on [128, 2NB] tiles ----
                    # V = (TI+1)*TG  (in place onto TI)
                    nc.vector.scalar_tensor_tensor(
                        TI[:], TI[:], 1.0, TG[:], op0=OP.add, op1=OP.mult)
                    C = TI
                    if not first:
                        nc.vector.scalar_tensor_tensor(
                            TF[:], TF[:], 1.0, d_prev[nt][:], op0=OP.add, op1=OP.mult)
                        nc.vector.tensor_add(C[:], C[:], TF[:])
                    tct = cpool.tile([128, 2 * NB], dt_c, name="tct", tag=f"tct{nt}")
                    nc.scalar.activation(tct[:], C[:], AF.Tanh, scale=0.5)
                    Ht = carry.tile([128, 2 * NB], dt_c, name="Ht", tag=f"Ht{nt}")
                    nc.vector.scalar_tensor_tensor(
                        Ht[:], TO[:], 1.0, tct[:], op0=OP.add, op1=OP.mult)
                    Dt = carry.tile([128, 2 * NB], dt_c, name="Dt", tag=f"Dt{nt}")
                    for ci in range(HC):
                        nc.vector.tensor_mul(Dt[:, ci * NB:(ci + 1) * NB], lam[:],
                                             C[:, ci * NB:(ci + 1) * NB])

                    # ---- y chain ----
                    ps1 = psum.tile([128, NB], f32, name="ps1", tag="sps", bufs=2)
                    for k in range(HC):
                        nc.tensor.matmul(ps1[:], s_w1[k][:],
                                         Ht[:, k * NB:(k + 1) * NB],
                                         start=(k == 0), stop=(k == HC - 1))
                    y1 = cpool.tile([128, NB], dt_mm, name="y1", tag=f"y1{nt}")
                    nc.vector.tensor_scalar(y1[:], ps1[:], s_bv[:, 0:1], 0.0,
                                            op0=OP.add, op1=OP.max)
                    ps2 = psum.tile([128, NB], f32, name="ps2", tag="sps", bufs=2)
                    nc.tensor.matmul(ps2[:], s_w2[:], y1[:], start=True, stop=True)
                    y2 = cpool.tile([128, NB], dt_mm, name="y2", tag=f"y2{nt}")
                    nc.vector.tensor_scalar(y2[:], ps2[:], s_bv[:, 1:2], 0.0,
                                            op0=OP.add, op1=OP.max)
                    ps3 = psum.tile([128, NB], f32, name="ps3", tag="sps", bufs=2)
                    nc.tensor.matmul(ps3[:], s_w3[:], y2[:], start=True, stop=True)
                    y_f32 = carry.tile([128, NB], f32, name="yf", tag=f"yf{nt}")
                    nc.scalar.activation(y_f32[:], ps3[:], AF.Relu, bias=s_bv[:, 2:3])
                    nc.sync.dma_start(dys[j, :, ns], y_f32[:])
                    # y for matmuls comes straight from psum (parallel w/ ACT evac)
                    y_t = carry.tile([128, NB], dt_mm, name="ymm", tag=f"ymm{nt}")
                    nc.vector.tensor_scalar(y_t[:], ps3[:], s_bv[:, 2:3], 0.0,
                                            op0=OP.add, op1=OP.max)

                    # ---- inference + prior (y contribution last) ----
                    ips = psum.tile([128, NB], f32, name="ips", tag="sps", bufs=2)
                    nc.tensor.matmul(ips[:], s_I[:], s_base47[:, ns],
                                     start=True, stop=False)
                    if not first:
                        nc.tensor.matmul(ips[:], s_W47yp[:], y_prev[nt][:],
                                         start=False, stop=False)
                    for k in range(HC):
                        nc.tensor.matmul(ips[:], s_W47c[k][:],
                                         C[:, k * NB:(k + 1) * NB],
                                         start=False, stop=False)
                    nc.tensor.matmul(ips[0:Z, :], s_W4y[:], y_t[:],
                                     start=False, stop=True)
                    hz = cpool.tile([128, NB], dt_mm, name="hz", tag=f"hz{nt}")
                    nc.scalar.activation(hz[:], ips[:], AF.Relu)

                    mps = psum.tile([128, NB], f32, name="mps", tag="sps", bufs=2)
                    nc.tensor.matmul(mps[:], s_W58[:], hz[:], start=True, stop=True)
                    mean = cpool.tile([128, NB], f32, name="mean", tag=f"mean{nt}")
                    nc.scalar.activation(mean[:], mps[:], AF.Relu, bias=s_bv[:, 3:4])
                    nc.sync.dma_start(dmeans[j, :, ns], mean[0:Z, :])

                    lps = psum.tile([128, NB], f32, name="lps", tag="sps", bufs=2)
                    nc.tensor.matmul(lps[:], s_W69[:], hz[:], start=True, stop=True)
                    lv_out = cpool.tile([Z, NB], f32, name="lvo", tag=f"lvo{nt}")
                    nc.vector.tensor_scalar(lv_out[:], lps[0:Z, :], s_bv[0:Z, 5:6],
                                            0.0, op0=OP.add, op1=OP.max)
                    nc.sync.dma_start(dlvs[j, :, ns], lv_out[:])
                    s_t = cpool.tile([128, NB], dt_c, name="st", tag=f"st{nt}")
                    nc.scalar.activation(s_t[:], lps[:], AF.Exp, scale=0.5,
                                         bias=s_bv[:, 4:5])

                    tns = cpool.tile([128, NB], dt_c, name="tns", tag=f"tns{nt}")
                    nc.vector.scalar_tensor_tensor(
                        tns[:], s_t[:], 1.0, eps[:], op0=OP.max, op1=OP.mult)
                    # z carry (bf16, rows 0:Z only) computed in parallel with the
                    # fp32 output add so the next step's gates unblock sooner
                    zmm = carry.tile([Z, NB], dt_mm, name="zmm", tag=f"zmm{nt}")
                    nc.vector.tensor_add(zmm[:], tns[0:Z, :], mean[0:Z, :])
                    zt = carry.tile([128, NB], f32, name="zt", tag=f"zt{nt}")
                    nc.vector.tensor_add(zt[:], tns[:], mean[:])
                    nc.sync.dma_start(dzs[j, :, ns], zt[0:Z, :])
                    nc.sync.dma_start(dzps[j, :, ns], zt[Z:128, :])

                    z_prev[nt] = zmm
                    y_prev[nt] = y_t
                    h_prev[nt] = Ht
                    d_prev[nt] = Dt

    nc.compile()
    return nc


def _prep_inputs(inputs, dt_mm_name, cell_dt_name):
    import ml_dtypes
    def to_dt(a, name):
        a = np.ascontiguousarray(a, np.float32)
        return a if name == "float32" else a.astype(ml_dtypes.bfloat16)
    cast = lambda a: to_dt(a, dt_mm_name)
    castc = lambda a: to_dt(a, cell_dt_name)

    h_i = np.asarray(inputs["h_i"], np.float32)
    input_t = np.asarray(inputs["input_t"], np.float32)
    eps_inf = np.asarray(inputs["eps_inf"], np.float32)
    eps_prior = np.asarray(inputs["eps_prior"], np.float32)
    Wx = np.asarray(inputs["Wx"], np.float32)
    Wh = np.asarray(inputs["Wh"], np.float32)
    b_lstm = np.asarray(inputs["b_lstm"], np.float32)
    ws = {i: np.asarray(inputs[f"w{i}"], np.float32) for i in range(1, 10)}
    bs = {i: np.asarray(inputs[f"b{i}"], np.float32) for i in range(1, 10)}
    alpha = float(np.asarray(inputs["alpha"]).reshape(-1)[0])
    beta = float(np.asarray(inputs["beta"]).reshape(-1)[0])
    mu0 = float(np.asarray(inputs["mu0"]).reshape(-1)[0])

    t = input_t
    j3 = np.arange(STEPS) + 3
    diff = t[:, None, :] - t[:, j3][:, :, None]               # [B, STEPS, T]
    mask = np.arange(T)[None, None, :] < j3[None, :, None]
    trig = np.where(mask, np.exp(diff.astype(np.float32)), 0.0).sum(axis=2)
    lam = 0.5 * (mu0 + alpha * beta * trig)
    lamT_full = np.ascontiguousarray(lam.T.astype(np.float32))  # [STEPS, B]

    W58 = np.zeros((128, 128), np.float32)
    W58[0:Z, 0:Z] = ws[5]; W58[Z:, Z:] = ws[8]
    W69 = np.zeros((128, 128), np.float32)
    W69[0:Z, 0:Z] = ws[6]; W69[Z:, Z:] = ws[9]

    baseg_full = (h_i @ Wx[Z:Z + H] + b_lstm).T.astype(np.float32)
    base47_full = (h_i @ np.concatenate([ws[4][0:H], ws[7][0:H]], axis=1)
                   + np.concatenate([bs[4], bs[7]])).T.astype(np.float32)

    bvec = np.zeros((128, 6), np.float32)
    bvec[:, 0] = bs[1]; bvec[:, 1] = bs[2]; bvec[:, 2] = bs[3]
    bvec[:, 3] = np.concatenate([bs[5], bs[8]])
    bvec[:, 4] = 0.5 * np.concatenate([bs[6], bs[9]])
    bvec[:, 5] = np.concatenate([bs[6], bs[9]])

    shared = dict(
        Wxz=cast(Wx[0:Z]),
        Wxy=cast(Wx[Z + H:]),
        Whh=cast(0.5 * Wh),
        w1h=cast(0.5 * ws[1]),
        w2=cast(ws[2]), w3=cast(ws[3]),
        W47c=cast(0.5 * np.concatenate([ws[4][H:2 * H], ws[7][H:2 * H]], axis=1)),
        W4y=cast(ws[4][2 * H:2 * H + F]),
        W47yp=cast(np.concatenate([ws[4][2 * H + F:], ws[7][2 * H:]], axis=1)),
        W58=cast(W58), W69=cast(W69),
        ident=cast(np.eye(128, dtype=np.float32)),
        bvec=bvec,
    )
    eps_full = np.concatenate(
        [eps_inf.transpose(0, 2, 1), eps_prior.transpose(0, 2, 1)], axis=1)

    in_maps = []
    for c in range(N_CORES):
        bsl = slice(c * NBC, (c + 1) * NBC)
        m = dict(shared)
        m["epsT"] = castc(eps_full[:, :, bsl])
        m["lamT"] = castc(lamT_full[:, bsl])
        m["baseg"] = cast(baseg_full[:, bsl])
        m["base47"] = cast(base47_full[:, bsl])
        in_maps.append(m)
    return in_maps


def _assemble(per_core_outs):
    outs = []
    for name in ["ys", "means", "lvs", "zs", "zps"]:
        full = np.concatenate([o[name] for o in per_core_outs], axis=2)  # [S, F, B]
        outs.append(np.ascontiguousarray(full.transpose(2, 0, 1)))
    return tuple(outs)


def get_nc():
    key = (DT_MM, CELL_DT, REPEAT)
    if key not in _cache:
        _cache[key] = _build_kernel(DT_MM, CELL_DT, REPEAT)
    return _cache[key]


def kernel(**inputs):
    from concourse.bass_utils import run_bass_kernel_spmd
    nc = get_nc()
    in_maps = _prep_inputs(inputs, DT_MM, CELL_DT)
    res = run_bass_kernel_spmd(nc, in_maps, core_ids=list(range(N_CORES)))
    return _assemble(res.results)
